# revision 4
# baseline (speedup 1.0000x reference)
"""Causal self-attention kernel for 8 Trainium2 NeuronCores.

Problem (hardcoded): x [4, 2048, 1024], torch-style Linear weights
W_q/W_k/W_v/W_o [1024, 1024], b_o [1024]; 16 heads, head_dim 64,
causal softmax attention, out = attn(x) @ W_o.T + b_o.

Sharding: 8 cores = 4 batches x 2 head-groups (8 heads each).
Each core computes a partial output  y_g @ W_o[:, g].T  for its batch;
the host sums the two head-group partials and adds b_o (unshard step).

Per-core pipeline (all matmuls on PE at 1 cycle/row):
  phase 1: QKV projections in float32r from xT [D, T] and pre-transposed
           weights; q/k written as qT/kT [dq, T] fp16, v as natural
           [T, dv] fp16 with an appended ones-column (v_aug).
  phase 2: per head: scores S^T[k, q] (K=64 matmuls) -> exp on ACT
           (scale=1/8) directly from PSUM into fp16 expP tiles (causal
           span only) -> causal masking via memset + triangular
           mask-multiply -> PV matmuls yT_aug[65, Tq] = v_aug.T @ expP;
           row 64 is the softmax denominator (ones-column trick);
           normalize yT via reciprocal + PE ones-broadcast + multiply.
  phase 3: out[T, D] = yT_norm.T @ W_o_g.T in fp16, fp32 out.
"""

import numpy as np

import concourse.bass as bass
import concourse.tile as tile
import concourse.mybir as mybir
from concourse import bacc
from concourse import bass_utils

T = 2048
D = 1024
HPC = 8            # heads per core
DH = 64
DQ = HPC * DH      # 512, per-core projection width
NT = T // 128      # 16 row tiles
NJ = DQ // 128     # 4 dq tiles
NC4 = T // 512     # 4 Tq chunks

F32 = mybir.dt.float32
F32R = mybir.dt.float32r
F16 = mybir.dt.float16
EXP = mybir.ActivationFunctionType.Exp

TRACE = False
LAST = None        # BassKernelResults of the most recent run

TRIMASK = np.triu(np.ones((128, 128), dtype=np.float16))


def _body(tc):
    nc = tc.nc
    xT_d = nc.dram_tensor("xt", (D, T), F32R, kind="ExternalInput").ap()
    wq_d = nc.dram_tensor("wqt", (D, DQ), F32R, kind="ExternalInput").ap()
    wk_d = nc.dram_tensor("wkt", (D, DQ), F32R, kind="ExternalInput").ap()
    wv_d = nc.dram_tensor("wvt", (D, DQ), F32R, kind="ExternalInput").ap()
    wo_d = nc.dram_tensor("wot", (DQ, D), F16, kind="ExternalInput").ap()
    tm_d = nc.dram_tensor("trimask", (128, 128), F16, kind="ExternalInput").ap()
    out_d = nc.dram_tensor("out", (T, D), F32, kind="ExternalOutput").ap()

    with (
        tc.tile_pool(name="persist", bufs=1) as pp,
        tc.tile_pool(name="psum_st", bufs=2, space="PSUM") as stp,
        tc.tile_pool(name="psum_gp", bufs=2, space="PSUM") as gpp,
        tc.tile_pool(name="psum_y", bufs=2, space="PSUM") as ypp,
    ):
        qT = pp.tile([128, NJ, T], F16, tag="qT")
        kT = pp.tile([128, NJ, T], F16, tag="kT")
        v = pp.tile([128, NT, HPC, DH + 1], F16, tag="v")
        yT = pp.tile([128, NJ, T], F16, tag="yT")
        woT = pp.tile([128, NJ, D], F16, tag="woT")
        trim = pp.tile([128, 128], F16, tag="trim")
        ones = pp.tile([1, DH], F16, tag="ones")

        nc.sync.dma_start(woT[:], wo_d.rearrange("(j p) n -> p j n", p=128))
        nc.sync.dma_start(trim[:], tm_d)
        nc.vector.memset(ones[:], 1.0)
        nc.vector.memset(v[:, :, :, DH:DH + 1], 1.0)

        # ---- phase 1: QKV projections (float32r) ----
        with (
            tc.tile_pool(name="xpool", bufs=1) as xp,
            tc.tile_pool(name="wpool", bufs=2) as wp,
        ):
            xts = []
            for k in range(8):
                xt = xp.tile([128, T], F32R, tag=f"x{k}")
                nc.sync.dma_start(xt[:], xT_d[128 * k:128 * (k + 1), :])
                xts.append(xt)

            for wdram, dest in ((wq_d, qT), (wk_d, kT)):
                wt = wp.tile([128, 8, DQ], F32R, tag="w")
                nc.sync.dma_start(wt[:], wdram.rearrange("(c p) n -> p c n", p=128))
                for j in range(NJ):
                    for c in range(NC4):
                        ps = gpp.tile([128, 512], F32, tag="g")
                        for k in range(8):
                            nc.tensor.matmul(
                                ps[:],
                                wt[:, k, 128 * j:128 * (j + 1)],
                                xts[k][:, 512 * c:512 * (c + 1)],
                                start=(k == 0), stop=(k == 7),
                            )
                        nc.vector.tensor_copy(dest[:, j, 512 * c:512 * (c + 1)], ps[:])

            wt = wp.tile([128, 8, DQ], F32R, tag="w")
            nc.sync.dma_start(wt[:], wv_d.rearrange("(c p) n -> p c n", p=128))
            for t in range(NT):
                ps = gpp.tile([128, 512], F32, tag="g")
                for k in range(8):
                    nc.tensor.matmul(
                        ps[:],
                        xts[k][:, 128 * t:128 * (t + 1)],
                        wt[:, k, :],
                        start=(k == 0), stop=(k == 7),
                    )
                nc.vector.tensor_copy(
                    v[:, t, :, 0:DH], ps[:].rearrange("p (h d) -> p h d", h=HPC)
                )

        # ---- phases 2+3 (pools opened after phase-1 pools release) ----
        with (
            tc.tile_pool(name="expp", bufs=2) as epool,
            tc.tile_pool(name="small", bufs=2) as sp,
            tc.tile_pool(name="outsb", bufs=2) as op,
        ):
            # ---- phase 2: attention per head ----
            for h in range(HPC):
                hp = (h % 2) * DH
                hj = h // 2
                expps = []
                for r in range(NT):
                    s0 = 512 * (r // 4)
                    ep = epool.tile([128, T - s0], F16, tag=f"e{r}")
                    expps.append((ep, s0))
                    if 128 * r > s0:
                        nc.vector.memset(ep[:, 0:128 * r - s0], 0.0)
                    bounds = [(s0, 1024), (1024, 2048)] if s0 < 1024 else [(s0, 2048)]
                    for (lo, hi) in bounds:
                        st = stp.tile([128, hi - lo], F32, tag="st")
                        for n0 in range(lo, hi, 512):
                            nc.tensor.matmul(
                                st[:, n0 - lo:n0 - lo + 512],
                                kT[hp:hp + DH, hj, 128 * r:128 * (r + 1)],
                                qT[hp:hp + DH, hj, n0:n0 + 512],
                                start=True, stop=True,
                            )
                        elo = max(lo, 128 * r)
                        nc.scalar.activation(
                            ep[:, elo - s0:hi - s0], st[:, elo - lo:hi - lo],
                            EXP, scale=0.125,
                        )
                    db = 128 * r - s0
                    nc.vector.tensor_mul(
                        ep[:, db:db + 128], ep[:, db:db + 128], trim[:]
                    )

                for c in range(NC4):
                    ya = ypp.tile([DH + 1, 512], F32, tag="y")
                    rmax = 4 * c + 3
                    for r in range(rmax + 1):
                        ep, s0 = expps[r]
                        nc.tensor.matmul(
                            ya[:], v[:, r, h, :],
                            ep[:, 512 * c - s0:512 * c - s0 + 512],
                            start=(r == 0), stop=(r == rmax),
                        )
                    yt = sp.tile([DH + 1, 512], F16, tag="yt")
                    nc.vector.tensor_copy(yt[:], ya[:])
                    rec = sp.tile([1, 512], F16, tag="rec")
                    with nc.allow_low_precision(reason="softmax reciprocal fp16"):
                        nc.vector.reciprocal(rec[:], ya[DH:DH + 1, :])
                    bc = ypp.tile([DH, 512], F32, tag="y")
                    nc.tensor.matmul(bc[:], ones[:], rec[:], start=True, stop=True)
                    bcs = sp.tile([DH, 512], F16, tag="bcs")
                    nc.vector.tensor_copy(bcs[:], bc[:])
                    nc.vector.tensor_mul(
                        yT[hp:hp + DH, hj, 512 * c:512 * (c + 1)], yt[0:DH, :], bcs[:]
                    )

            # ---- phase 3: output projection (fp16) ----
            for i in range(NT):
                ob = op.tile([128, D], F32, tag="ob")
                for d in range(2):
                    ps = gpp.tile([128, 512], F32, tag="g")
                    for j in range(NJ):
                        nc.tensor.matmul(
                            ps[:],
                            yT[:, j, 128 * i:128 * (i + 1)],
                            woT[:, j, 512 * d:512 * (d + 1)],
                            start=(j == 0), stop=(j == 3),
                        )
                    nc.vector.tensor_copy(ob[:, 512 * d:512 * (d + 1)], ps[:])
                nc.sync.dma_start(out_d[128 * i:128 * (i + 1), :], ob[:])


def build_nc():
    nc = bacc.Bacc("TRN2", target_bir_lowering=False, debug=False)
    with tile.TileContext(nc) as tc:
        _body(tc)
    nc.compile()
    return nc


_nc_cache = None


def _get_nc():
    global _nc_cache
    if _nc_cache is None:
        _nc_cache = build_nc()
    return _nc_cache


def make_in_maps(x, W_q, W_k, W_v, W_o):
    x = np.asarray(x, dtype=np.float32)
    W_q = np.asarray(W_q, dtype=np.float32)
    W_k = np.asarray(W_k, dtype=np.float32)
    W_v = np.asarray(W_v, dtype=np.float32)
    W_o = np.asarray(W_o, dtype=np.float32)
    in_maps = []
    for c in range(8):
        b, g = divmod(c, 2)
        sl = slice(DQ * g, DQ * (g + 1))
        in_maps.append({
            "xt": np.ascontiguousarray(x[b].T),
            "wqt": np.ascontiguousarray(W_q[sl].T),
            "wkt": np.ascontiguousarray(W_k[sl].T),
            "wvt": np.ascontiguousarray(W_v[sl].T),
            "wot": np.ascontiguousarray(W_o[:, sl].T).astype(np.float16),
            "trimask": TRIMASK,
        })
    return in_maps


def kernel(x, W_q, W_k, W_v, W_o, b_o):
    global LAST
    nc = _get_nc()
    in_maps = make_in_maps(x, W_q, W_k, W_v, W_o)
    res = bass_utils.run_bass_kernel_spmd(
        nc, in_maps, core_ids=list(range(8)), trace=TRACE
    )
    LAST = res
    parts = [res.results[c]["out"] for c in range(8)]
    b_o = np.asarray(b_o, dtype=np.float32)
    out = np.stack([parts[2 * b] + parts[2 * b + 1] for b in range(4)])
    out += b_o[None, None, :]
    return out.astype(np.float32)


# revision 26
# speedup vs baseline: 1.2117x; 1.2117x over previous
"""Causal self-attention kernel for 8 Trainium2 NeuronCores.

Problem (hardcoded): x [4, 2048, 1024], torch-style Linear weights
W_q/W_k/W_v/W_o [1024, 1024], b_o [1024]; 16 heads, head_dim 64,
causal softmax attention, out = attn(x) @ W_o.T + b_o.

Sharding: 8 cores = 4 batches x 2 head-groups (8 heads each).
Each core computes a partial output  y_g @ W_o[:, g].T  for its batch;
the host sums the two head-group partials and adds b_o (unshard step).

Per-core pipeline (all matmuls on PE at 1 cycle/row):
  phase 1: QKV projections in float32r from xT [D, T] and pre-transposed
           weights; q/k written as qT/kT [dq, T] fp16, v as natural
           [T, dv] fp16 with an appended ones-column (v_aug).
  phase 2: per head: scores S^T[k, q] (K=64 matmuls) -> exp on ACT
           (scale=1/8) directly from PSUM into fp16 expP tiles (causal
           span only) -> causal masking via memset + triangular
           mask-multiply -> PV matmuls yT_aug[65, Tq] = v_aug.T @ expP;
           row 64 is the softmax denominator (ones-column trick);
           normalize yT via reciprocal + PE ones-broadcast + multiply.
  phase 3: out[T, D] = yT_norm.T @ W_o_g.T in fp16, fp32 out.
"""

import numpy as np

import concourse.bass as bass
import concourse.tile as tile
import concourse.mybir as mybir
from concourse import bacc
from concourse import bass_utils

T = 2048
D = 1024
HPC = 8            # heads per core
DH = 64
DQ = HPC * DH      # 512, per-core projection width
NT = T // 128      # 16 row tiles
NJ = DQ // 128     # 4 dq tiles
NC4 = T // 512     # 4 Tq chunks

F32 = mybir.dt.float32
F32R = mybir.dt.float32r
F16 = mybir.dt.float16
EXP = mybir.ActivationFunctionType.Exp

TRACE = False
LAST = None        # BassKernelResults of the most recent run

TRIMASK = np.triu(np.ones((128, 128), dtype=np.float16))


def _body(tc):
    nc = tc.nc
    xT_d = nc.dram_tensor("xt", (D, T), F32R, kind="ExternalInput").ap()
    wq_d = nc.dram_tensor("wqt", (D, DQ), F32R, kind="ExternalInput").ap()
    wk_d = nc.dram_tensor("wkt", (D, DQ), F32R, kind="ExternalInput").ap()
    # fp16 copy of x, host-laid as [p, t, k, col] for the v projection
    xv_d = nc.dram_tensor("xv", (128, NT, 8, 128), F16, kind="ExternalInput").ap()
    wv_d = nc.dram_tensor("wvt", (D, DQ), F16, kind="ExternalInput").ap()
    wo_d = nc.dram_tensor("wot", (DQ, D), F16, kind="ExternalInput").ap()
    tm_d = nc.dram_tensor("trimask", (128, 128), F16, kind="ExternalInput").ap()
    out_d = nc.dram_tensor("out", (T, D), F32, kind="ExternalOutput").ap()

    with (
        tc.tile_pool(name="persist", bufs=1) as pp,
        tc.tile_pool(name="psum_y", bufs=2, space="PSUM") as ypp,
    ):
        qT = pp.tile([128, NJ, T], F16, tag="qT")
        kT = pp.tile([128, NJ, T], F16, tag="kT")
        v = pp.tile([128, NT, HPC, DH + 1], F16, tag="v")
        yT = pp.tile([128, NJ, T], F16, tag="yT")
        woT = pp.tile([128, NJ, D], F16, tag="woT")
        trim = pp.tile([128, 128], F16, tag="trim")
        ones = pp.tile([1, DH], F16, tag="ones")

        nc.gpsimd.memset(ones[:], 1.0)
        nc.gpsimd.memset(v[:, :, :, DH:DH + 1], 1.0)
        # warm the ACT exp table while DMAs run
        warm = pp.tile([1, DH], F16, tag="warm")
        nc.scalar.activation(warm[:], ones[:], EXP, scale=1.0)

        # ---- phase 1a: Q/K projections (float32r) ----
        # DMA issue order matters: wq first so the first matmul can start
        # ~4us in; inputs needed later go last on the queue.
        with (
            tc.tile_pool(name="xpool", bufs=1) as xp,
            tc.tile_pool(name="wpool", bufs=2) as wp,
            tc.tile_pool(name="psum_q", bufs=6, space="PSUM") as qpp,
        ):
            wts = [wp.tile([128, 8, DQ], F32R, tag="w", name=f"w{i}")
                   for i in range(2)]
            xts = [xp.tile([128, T], F32R, tag=f"x{k}", name=f"xt_{k}")
                   for k in range(8)]

            def _wload(i, half):
                wsrc = (wq_d, wk_d)[i].rearrange("(c p) n -> p c n", p=128)
                nc.sync.dma_start(wts[i][:, 4 * half:4 * half + 4, :],
                                  wsrc[:, 4 * half:4 * half + 4, :])

            # split x loads across the HWDGE (sync) and SWDGE (gpsimd)
            # queues so they land in parallel
            _wload(0, 0)
            _wload(0, 1)
            for k in (1, 3, 5, 7):
                nc.gpsimd.dma_start(xts[k][:], xT_d[128 * k:128 * (k + 1), :])
            for k in (0, 2, 4, 6):
                nc.sync.dma_start(xts[k][:], xT_d[128 * k:128 * (k + 1), :])
            _wload(1, 0)
            _wload(1, 1)
            nc.sync.dma_start(trim[:], tm_d)

            # contraction in approximate DMA-arrival order
            KS = (1, 3, 0, 5, 2, 7, 4, 6)
            for wt, dest in ((wts[0], qT), (wts[1], kT)):
                for j in range(NJ):
                    for c in range(NC4):
                        ps = qpp.tile([128, 512], F32, tag="q")
                        for ki, k in enumerate(KS):
                            nc.tensor.matmul(
                                ps[:],
                                wt[:, k, 128 * j:128 * (j + 1)],
                                xts[k][:, 512 * c:512 * (c + 1)],
                                start=(ki == 0), stop=(ki == 7),
                            )
                        nc.vector.tensor_copy(dest[:, j, 512 * c:512 * (c + 1)], ps[:])

        # ---- phases 1b+2+3: v-projection (fp16) + attention + out-proj ----
        with (
            tc.tile_pool(name="xvpool", bufs=4) as xvp,
            tc.tile_pool(name="wvpool", bufs=1) as wvp,
            tc.tile_pool(name="psum_st", bufs=2, space="PSUM") as stp,
            tc.tile_pool(name="psum_g", bufs=2, space="PSUM") as gpp,
            tc.tile_pool(name="expp", bufs=2) as epool,
            tc.tile_pool(name="small", bufs=2) as sp,
            tc.tile_pool(name="outsb", bufs=2) as op,
        ):
            wv16 = wvp.tile([128, 8, DQ], F16, tag="wv")
            wvsrc = wv_d.rearrange("(c p) n -> p c n", p=128)
            nc.sync.dma_start(wv16[:], wvsrc)
            xv_tiles = [xvp.tile([128, 8, 128], F16, tag="xv", name=f"xv_{t}")
                        for t in range(NT)]
            for t in range(NT):
                nc.sync.dma_start(xv_tiles[t][:], xv_d[:, t, :, :])
            nc.sync.dma_start(woT[:], wo_d.rearrange("(j p) n -> p j n", p=128))

            def scores_head(h):
                hp = (h % 2) * DH
                hj = h // 2
                expps = []
                for r in range(NT):
                    s0 = 512 * (r // 4)
                    ep = epool.tile([128, T - s0], F16, tag=f"e{r}",
                                    name=f"e{r}_h{h}")
                    expps.append((ep, s0))
                    if 128 * r > s0:
                        nc.gpsimd.memset(ep[:, 0:128 * r - s0], 0.0)
                    bounds = ([(s0, 1024), (1024, 2048)] if s0 < 1024
                              else [(s0, 2048)])
                    for (lo, hi) in bounds:
                        st = stp.tile([128, hi - lo], F32, tag="st")
                        for n0 in range(lo, hi, 512):
                            c0 = max(n0, 128 * r)  # exact-causal start
                            nc.tensor.matmul(
                                st[:, c0 - lo:n0 - lo + 512],
                                kT[hp:hp + DH, hj, 128 * r:128 * (r + 1)],
                                qT[hp:hp + DH, hj, c0:n0 + 512],
                                start=True, stop=True,
                            )
                        elo = max(lo, 128 * r)
                        nc.scalar.activation(
                            ep[:, elo - s0:hi - s0], st[:, elo - lo:hi - lo],
                            EXP, scale=0.125,
                        )
                    db = 128 * r - s0
                    nc.gpsimd.tensor_mul(
                        ep[:, db:db + 128], ep[:, db:db + 128], trim[:]
                    )
                return expps

            def pv_head(h, expps):
                hp = (h % 2) * DH
                hj = h // 2
                for c in range(NC4):
                    ya = ypp.tile([DH + 1, 512], F32, tag="y")
                    rmax = 4 * c + 3
                    for r in range(rmax + 1):
                        ep, s0 = expps[r]
                        off = max(0, 128 * r - 512 * c)  # exact-causal start
                        nc.tensor.matmul(
                            ya[:, off:512], v[:, r, h, :],
                            ep[:, 512 * c - s0 + off:512 * c - s0 + 512],
                            start=(r == 0), stop=(r == rmax),
                        )
                    yt = sp.tile([DH + 1, 512], F16, tag="yt")
                    nc.vector.tensor_copy(yt[:], ya[:])
                    rec = sp.tile([1, 512], F16, tag="rec")
                    with nc.allow_low_precision(reason="softmax reciprocal fp16"):
                        nc.vector.reciprocal(rec[:], ya[DH:DH + 1, :])
                    bc = ypp.tile([DH, 512], F32, tag="y")
                    nc.tensor.matmul(bc[:], ones[:], rec[:], start=True, stop=True)
                    bcs = sp.tile([DH, 512], F16, tag="bcs")
                    nc.vector.tensor_copy(bcs[:], bc[:])
                    nc.gpsimd.tensor_mul(
                        yT[hp:hp + DH, hj, 512 * c:512 * (c + 1)], yt[0:DH, :], bcs[:]
                    )

            def vproj(t0, t1):
                for t in range(t0, t1):
                    ps = gpp.tile([128, 512], F32, tag="g")
                    for k in range(8):
                        nc.tensor.matmul(
                            ps[:],
                            xv_tiles[t][:, k, :],
                            wv16[:, k, :],
                            start=(k == 0), stop=(k == 7),
                        )
                    nc.vector.tensor_copy(
                        v[:, t, :, 0:DH], ps[:].rearrange("p (h d) -> p h d", h=HPC)
                    )

            # Software-pipelined heads: scores for h0+h1 first so ACT gets
            # continuous work, fp16 v-projection splits around them on PE,
            # then scores(h+1) is emitted ahead of pv(h) throughout.
            expps = {0: scores_head(0)}
            vproj(0, 9)
            expps[1] = scores_head(1)
            vproj(9, NT)
            for h in range(HPC):
                if h + 2 < HPC:
                    expps[h + 2] = scores_head(h + 2)
                pv_head(h, expps.pop(h))

            # ---- phase 3: output projection (fp16) ----
            for i in range(NT):
                ob = op.tile([128, D], F32, tag="ob")
                for d in range(2):
                    ps = gpp.tile([128, 512], F32, tag="g")
                    for j in range(NJ):
                        nc.tensor.matmul(
                            ps[:],
                            yT[:, j, 128 * i:128 * (i + 1)],
                            woT[:, j, 512 * d:512 * (d + 1)],
                            start=(j == 0), stop=(j == 3),
                        )
                    nc.scalar.copy(ob[:, 512 * d:512 * (d + 1)], ps[:])
                nc.sync.dma_start(out_d[128 * i:128 * (i + 1), :], ob[:])


def build_nc():
    nc = bacc.Bacc("TRN2", target_bir_lowering=False, debug=False)
    with tile.TileContext(nc) as tc:
        _body(tc)
    nc.compile()
    return nc


_nc_cache = None


def _get_nc():
    global _nc_cache
    if _nc_cache is None:
        _nc_cache = build_nc()
    return _nc_cache


def make_in_maps(x, W_q, W_k, W_v, W_o):
    x = np.asarray(x, dtype=np.float32)
    W_q = np.asarray(W_q, dtype=np.float32)
    W_k = np.asarray(W_k, dtype=np.float32)
    W_v = np.asarray(W_v, dtype=np.float32)
    W_o = np.asarray(W_o, dtype=np.float32)
    in_maps = []
    for c in range(8):
        b, g = divmod(c, 2)
        sl = slice(DQ * g, DQ * (g + 1))
        xT = np.ascontiguousarray(x[b].T)
        # [p, t, k, col] layout for the fp16 v-projection streaming tiles
        xv = np.ascontiguousarray(
            xT.astype(np.float16).reshape(8, 128, NT, 128).transpose(1, 2, 0, 3)
        )
        in_maps.append({
            "xt": xT,
            "xv": xv,
            "wqt": np.ascontiguousarray(W_q[sl].T),
            "wkt": np.ascontiguousarray(W_k[sl].T),
            "wvt": np.ascontiguousarray(W_v[sl].T).astype(np.float16),
            "wot": np.ascontiguousarray(W_o[:, sl].T).astype(np.float16),
            "trimask": TRIMASK,
        })
    return in_maps


def kernel(x, W_q, W_k, W_v, W_o, b_o):
    global LAST
    nc = _get_nc()
    in_maps = make_in_maps(x, W_q, W_k, W_v, W_o)
    res = bass_utils.run_bass_kernel_spmd(
        nc, in_maps, core_ids=list(range(8)), trace=TRACE
    )
    LAST = res
    parts = [res.results[c]["out"] for c in range(8)]
    b_o = np.asarray(b_o, dtype=np.float32)
    out = np.stack([parts[2 * b] + parts[2 * b + 1] for b in range(4)])
    out += b_o[None, None, :]
    return out.astype(np.float32)
